# revision 1
# baseline (speedup 1.0000x reference)
"""Trainium2 Bass kernel for DirectVolumeRenderer (axis-aligned camera).

Factorization (per depth p, camera R=I so sample coords are separable):
    ix(px) = a_p + s_p*px ; iy(py) = a_p + s_p*py ; iz = const(p)
    trilinear(vol) = z-lerp (2 slices, scalar weights) -> two matmuls with the
    SAME tent matrix  A_p[v,q] = relu(1 - |v - (a_p + s_p*q)|):
        T1   = Zp^T @ A_p          (contract y; PE "transposes" for free)
        feat = A_p^T @ T1          (contract x) -> image in [px,py] layout
    sigma_p = 0.1*az_p * av_p[px] (x) av_p[py]  (rank-1, host vectors)
    compositing (front-to-back): q=-sigma*Tacc; rgb+=-q*feat; Tacc+=q
    (in fp32 the reference's (1+1e-10) rounds to exactly 1.0)

Sharding: 240 active depths split into 8 contiguous runs of 30; per-core
Tacc entering each run is pure geometry -> host precomputes it. Cross-core
combine is a single 256KB AllReduce(sum) + (redundant) normalization.

Engines: PE does all matmuls + sigma outer-products + Tacc/rgb PSUM
accumulation (via identity matmuls); ACT builds |D| and the tent; GPSIMD
does the z-lerp; DVE does the two compositing multiplies + PSUM->SBUF moves.
"""
import os
import sys
import numpy as np

for _p in ("/opt/trn_rl_repo", "/root/.axon_site/_ro/trn_rl_repo"):
    if os.path.isdir(_p) and _p not in sys.path:
        sys.path.insert(0, _p)

IMG = 256
NPTS = 320
MIN_D, MAX_D = 2.0, 6.0
FOCAL = 2.0
DENSITY = 0.1
EPS = 1e-8
N_CORES = 8


# ----------------------------------------------------------------------------
# host-side geometry
# ----------------------------------------------------------------------------

def _geometry(T):
    """Per-depth separable sampling params (f64). Requires R=I and Tx==Ty."""
    Tx, Ty, Tz = float(T[0]), float(T[1]), float(T[2])
    vox = 3.0 / 256.0
    half = vox * 255.0 * 0.5
    depths = np.linspace(MIN_D, MAX_D, NPTS)
    c = depths * 127.5 / (2.0 * half)
    s = c * (2.0 / 255.0)
    a = 127.5 - c - Tx * 127.5 / half
    iz = 127.5 * ((depths - Tz) / half + 1.0)
    z0 = np.floor(iz).astype(np.int64)
    fz = iz - z0
    z1 = z0 + 1
    wz0 = np.where((z0 >= 0) & (z0 < 256), 1.0 - fz, 0.0)
    wz1 = np.where((z1 >= 0) & (z1 < 256), fz, 0.0)
    az = wz0 + wz1
    q = np.arange(IMG)
    ic = a[:, None] + s[:, None] * q[None, :]
    c0 = np.floor(ic)
    fc = ic - c0
    av = (np.where((c0 >= 0) & (c0 < 256), 1.0 - fc, 0.0)
          + np.where((c0 + 1 >= 0) & (c0 + 1 < 256), fc, 0.0))
    return dict(s=s, a=a, z0=z0, z1=z1, wz0=wz0, wz1=wz1, az=az, av=av,
                active=az > 0)


def _host_inputs(vol, T):
    """Build the 8 per-core input maps. vol: (256,256,256) f32 (z,y,x)."""
    import ml_dtypes
    bf16 = ml_dtypes.bfloat16
    g = _geometry(T)
    act = np.nonzero(g["active"])[0]
    nd = int(np.ceil(len(act) / N_CORES))

    # simulate the device's f32 Tacc recurrence to get per-core init tiles
    uneg_all = (-DENSITY * g["az"][:, None] * g["av"]).astype(np.float32)  # (P,256)
    v_all = g["av"].astype(np.float32)
    tacc = np.ones((IMG, IMG), np.float32)  # [px, py]
    vol16 = vol.astype(bf16)
    in_maps = []
    for cidx in range(N_CORES):
        ks = [int(act[i]) for i in range(cidx * nd, min((cidx + 1) * nd, len(act)))]

        slices = np.zeros((128, nd, 1024), bf16)
        tb = np.zeros((128, 2 * nd), np.float32)
        tsc = np.zeros((128, nd), np.float32)
        wzp = np.zeros((128, 2 * nd), np.float32)
        ut2 = np.zeros((2, 128 * nd), np.float32)
        vt2 = np.zeros((2, 512 * nd), np.float32)
        prow = np.arange(128, dtype=np.float32)

        for j, p in enumerate(ks):
            for si, zz in ((0, g["z0"][p]), (1, g["z1"][p])):
                sl = vol16[min(max(int(zz), 0), 255)]          # (y=256, x=256)
                # slab layout [part p, k*1024 + s*512 + yb*256 + x]
                slices[:, j, si * 512:(si + 1) * 512] = \
                    sl.reshape(2, 128, 256).transpose(1, 0, 2).reshape(128, 512)
            tsc[:, j] = np.float32(-g["s"][p])
            for b in (0, 1):
                tb[:, 2 * j + b] = (b * 128 + prow) - np.float32(g["a"][p])
                ut2[b, 128 * j:128 * (j + 1)] = uneg_all[p][128 * b:128 * (b + 1)]
                vt2[b, 512 * j + 256 * b:512 * j + 256 * (b + 1)] = v_all[p]
            wzp[:, 2 * j + 0] = np.float32(g["wz0"][p])
            wzp[:, 2 * j + 1] = np.float32(g["wz1"][p])

        # merged [p, b*256+py] layout of the Tacc tile entering this core
        t0 = np.ascontiguousarray(
            tacc.reshape(2, 128, IMG).transpose(1, 0, 2).reshape(128, 512))
        # advance the global f32 Tacc chain exactly as the device will
        for p in ks:
            sig = (-uneg_all[p][:, None]) * v_all[p][None, :]
            qv = (sig * tacc).astype(np.float32)
            tacc = (tacc - qv).astype(np.float32)

        pyio = np.broadcast_to(np.arange(256, dtype=np.float32), (128, 256)).copy()
        in_maps.append({
            "slices": slices.reshape(128, nd * 1024), "tb": tb, "tsc": tsc, "wzp": wzp,
            "ut2": ut2, "vt2": vt2, "tacc0": t0, "pyio": pyio,
            "ident": np.eye(128, dtype=np.float32),
        })
    return in_maps, nd


# ----------------------------------------------------------------------------
# device program
# ----------------------------------------------------------------------------

_NC_CACHE = {}


def _build_nc(nd, sim=False, repeat=1, ablate=()):
    """sim=True replaces the AllReduce with a local DMA copy so the
    single-core TimelineSim cost model can run the program. repeat>1
    re-runs the depth loop (garbage numerics) for slope-based timing.
    ablate: subset of {'zmerge','tent','mm','sigma','composit'} for
    timing ablations (wrong numerics)."""
    import concourse.bass as bass
    import concourse.tile as tile
    from concourse import bacc, mybir
    from contextlib import ExitStack

    dt = mybir.dt.float32
    dh = mybir.dt.bfloat16
    AF = mybir.ActivationFunctionType
    ALU = mybir.AluOpType

    nc = bacc.Bacc(None, num_devices=N_CORES)
    slices = nc.dram_tensor("slices", [128, nd * 1024], dh, kind="ExternalInput")
    tb_d = nc.dram_tensor("tb", [128, 2 * nd], dt, kind="ExternalInput")
    tsc_d = nc.dram_tensor("tsc", [128, nd], dt, kind="ExternalInput")
    wzp_d = nc.dram_tensor("wzp", [128, 2 * nd], dt, kind="ExternalInput")
    ut_d = nc.dram_tensor("ut2", [2, 128 * nd], dt, kind="ExternalInput")
    vt_d = nc.dram_tensor("vt2", [2, 512 * nd], dt, kind="ExternalInput")
    tacc0_d = nc.dram_tensor("tacc0", [128, 512], dt, kind="ExternalInput")
    pyio_d = nc.dram_tensor("pyio", [128, 256], dt, kind="ExternalInput")
    id_d = nc.dram_tensor("ident", [128, 128], dt, kind="ExternalInput")
    nrep_d = nc.dram_tensor("nrep", [1, 1], mybir.dt.int32, kind="ExternalInput")
    out_d = nc.dram_tensor("out", [256, 256], dt, kind="ExternalOutput")
    cc_in = nc.dram_tensor("cc_in", [256, 256], dt)
    cc_out = nc.dram_tensor("cc_out", [256, 256], dt, addr_space="Shared")

    with tile.TileContext(nc) as tc, ExitStack() as ctx:
        const = ctx.enter_context(tc.tile_pool(name="const", bufs=1))
        slp = ctx.enter_context(tc.tile_pool(name="slp", bufs=1))
        work = ctx.enter_context(tc.tile_pool(name="work", bufs=6))
        epil = ctx.enter_context(tc.tile_pool(name="epil", bufs=1))
        psum = ctx.enter_context(
            tc.tile_pool(name="psum", bufs=2, space=bass.MemorySpace.PSUM))
        pst1 = ctx.enter_context(
            tc.tile_pool(name="pst1", bufs=3, space=bass.MemorySpace.PSUM))
        psacc = ctx.enter_context(
            tc.tile_pool(name="psacc", bufs=1, space=bass.MemorySpace.PSUM))

        def cload(dram, shape):
            t = const.tile(shape, dt, tag=dram.name)
            nc.sync.dma_start(t[:], dram[:])
            return t

        tb = cload(tb_d, [128, 2 * nd])
        tsc = cload(tsc_d, [128, nd])
        wzp = cload(wzp_d, [128, 2 * nd])
        ut = cload(ut_d, [2, 128 * nd])
        vt = cload(vt_d, [2, 512 * nd])
        tacc0 = cload(tacc0_d, [128, 512])
        pyio = cload(pyio_d, [128, 256])
        ident = cload(id_d, [128, 128])

        # preload all slice pairs: one 512KB DMA per 2 depths (4KB/partition)
        slab = []
        for j in range((nd + 1) // 2):
            t = slp.tile([128, min(2048, (nd - 2 * j) * 1024)], dh, tag=f"slab{j}")
            nc.sync.dma_start(t[:], slices[:, j * 2048:j * 2048 + t.shape[1]])
            slab.append(t)

        zm_s = None
        at_s = None
        if "zmerge" in ablate or "mm" in ablate:
            zm_s = const.tile([128, 512], dh, tag="zm_s")
            nc.vector.tensor_copy(zm_s[:], slab[0][:, 0:512])
        if "tent" in ablate:
            at_s = const.tile([128, 512], dh, tag="at_s")
            nc.vector.tensor_copy(at_s[:], slab[0][:, 0:512])

        rgbps = psacc.tile([128, 512], dt, tag="rgb")
        # Tacc ping-pong tiles in SBUF, updated by DVE (keeps the serial
        # compositing chain entirely on one engine)
        gam0 = const.tile([128, 512], dt, tag="gam0")
        gam1 = const.tile([128, 512], dt, tag="gam1")
        gam = [gam0, gam1]
        nc.vector.tensor_copy(gam0[:], tacc0[:])
        zsb = const.tile([128, 512], dt, tag="zsb")
        nc.vector.memset(zsb[:], 0.0)
        nc.tensor.matmul(rgbps[:], ident[:], zsb[:], start=True, stop=False,
                         skip_group_check=True)
        nrep_t = const.tile([1, 1], mybir.dt.int32, tag="nrep")
        nc.sync.dma_start(nrep_t[:], nrep_d[:])
        import concourse.bass as _bass
        nregs = []
        for e in mybir.ALL_ENGINES:
            r = nc.engines[e].alloc_register(f"nrep_{e.name}")
            nc.engines[e].reg_load(r, nrep_t[0:1, 0:1])
            nregs.append(r)
        nrep_rh = _bass.RegisterHandles(nregs)

        with tc.For_i(0, nrep_rh, 1, hint_engines=(mybir.EngineType.PE,)):
          for k in range(nd):
              base = (k % 2) * 1024
              sl0 = slab[k // 2][:, base:base + 512]
              sl1 = slab[k // 2][:, base + 512:base + 1024]
              g0 = gam[k % 2]
              g1 = gam[(k + 1) % 2]

              # --- sigma / transmittance chain first (decoupled from feat) ---
              qp = None
              if "composit" not in ablate:
                  vbps = psum.tile([128, 512], dt, tag="vb")
                  nc.tensor.matmul(vbps[:],
                                   ut[:, 128 * k:128 * (k + 1)],
                                   vt[:, 512 * k:512 * (k + 1)],
                                   start=True, stop=True)
                  qp = work.tile([128, 512], dt, tag="qp")
                  nc.vector.tensor_mul(qp[:], vbps[:], g0[:])
                  nc.vector.tensor_add(g1[:], g0[:], qp[:])

              # --- z-lerp on DVE (bf16): zm = wz0*S0 + wz1*S1 ---
              if "zmerge" in ablate:
                  zm = zm_s
              else:
                  zm = work.tile([128, 512], dh, tag="zm")
                  zt = work.tile([128, 512], dh, tag="zt")
                  nc.vector.tensor_scalar_mul(zt[:], sl0, wzp[:, 2 * k:2 * k + 1])
                  nc.vector.scalar_tensor_tensor(
                      zm[:], sl1, wzp[:, 2 * k + 1:2 * k + 2], zt[:],
                      ALU.mult, ALU.add)

              # --- tent matrix A[p, B*256+q] = relu(1-|B*128+p - (a+s*q)|) ---
              if "tent" in ablate:
                  at = at_s
              else:
                  dab = work.tile([128, 512], dh, tag="dab")
                  for b in (0, 1):
                      nc.scalar.activation(dab[:, 256 * b:256 * (b + 1)], pyio[:],
                                           AF.Abs, bias=tb[:, 2 * k + b:2 * k + b + 1],
                                           scale=tsc[:, k:k + 1])
                  at = work.tile([128, 512], dh, tag="at")
                  nc.scalar.activation(at[:], dab[:], AF.Relu, bias=1.0, scale=-1.0)

              if "mm" in ablate:
                  continue
              # --- mm1: T1[x,py] = sum_y Zp[y,x] * A[y,py] ---
              t1ps = pst1.tile([128, 512], dt, tag="t1")
              for xc in (0, 1):
                  for yb in (0, 1):
                      nc.tensor.matmul(
                          t1ps[:, 256 * xc:256 * (xc + 1)],
                          zm[:, 256 * yb + 128 * xc:256 * yb + 128 * xc + 128],
                          at[:, 256 * yb:256 * (yb + 1)],
                          start=(yb == 0), stop=(yb == 1))
              t1sb = work.tile([128, 512], dh, tag="t1sb")
              nc.scalar.activation(t1sb[:], t1ps[:], AF.Copy, scale=-1.0)  # -T1

              # --- mm2: -feat[px,py] = sum_x A[x,px] * (-T1[x,py]) ---
              featps = psum.tile([128, 512], dt, tag="feat")
              for mb in (0, 1):
                  for xb in (0, 1):
                      nc.tensor.matmul(
                          featps[:, 256 * mb:256 * (mb + 1)],
                          at[:, 256 * xb + 128 * mb:256 * xb + 128 * mb + 128],
                          t1sb[:, 256 * xb:256 * (xb + 1)],
                          start=(xb == 0), stop=(xb == 1))

              if "composit" in ablate:
                  continue
              # --- weighted accumulation: rgb += (-q) * (-feat) ---
              wf = work.tile([128, 512], dt, tag="wf")
              nc.vector.tensor_mul(wf[:], qp[:], featps[:])
              nc.tensor.matmul(rgbps[:], ident[:], wf[:], start=False,
                               stop=False, skip_group_check=True)

        nc.tensor.matmul(rgbps[:], ident[:], zsb[:], start=False, stop=True,
                         skip_group_check=True)

        # ---- cross-core reduce + normalization ----
        rgbsb = epil.tile([128, 512], dt, tag="rgbsb")
        nc.vector.tensor_copy(rgbsb[:], rgbps[:])
        nc.sync.dma_start(cc_in[:].rearrange("(b p) y -> p b y", p=128),
                          rgbsb[:].rearrange("p (b y) -> p b y", b=2))
        if sim:
            nc.sync.dma_start(cc_out[:], cc_in[:])
        else:
            nc.gpsimd.collective_compute(
                "AllReduce", ALU.add, replica_groups=[list(range(N_CORES))],
                ins=[cc_in[:]], outs=[cc_out[:]])
        rgbf = epil.tile([128, 512], dt, tag="rgbf")
        nc.sync.dma_start(rgbf[:].rearrange("p (b y) -> p b y", b=2),
                          cc_out[:].rearrange("(b p) y -> p b y", p=128))

        from concourse import bass_isa
        sq = epil.tile([128, 512], dt, tag="sq")
        nc.vector.tensor_mul(sq[:], rgbf[:], rgbf[:])
        r4 = epil.tile([128, 4], dt, tag="r4")
        AX = mybir.AxisListType.X
        nc.vector.tensor_reduce(r4[:, 0:1], rgbf[:], axis=AX, op=ALU.min)
        nc.vector.tensor_reduce(r4[:, 1:2], rgbf[:], axis=AX, op=ALU.max)
        nc.vector.tensor_reduce(r4[:, 2:3], rgbf[:], axis=AX, op=ALU.add)
        nc.vector.tensor_reduce(r4[:, 3:4], sq[:], axis=AX, op=ALU.add)
        nc.vector.tensor_scalar_mul(r4[:, 0:1], r4[:, 0:1], -1.0)   # -min
        pr = epil.tile([128, 4], dt, tag="pr")
        nc.gpsimd.partition_all_reduce(pr[:, 0:2], r4[:, 0:2], 128,
                                       bass_isa.ReduceOp.max)
        nc.gpsimd.partition_all_reduce(pr[:, 2:4], r4[:, 2:4], 128,
                                       bass_isa.ReduceOp.add)
        # pr columns (on every partition): 0=-min 1=max 2=sum 3=sumsq
        n = float(IMG * IMG)
        w = epil.tile([128, 8], dt, tag="w")
        nc.vector.tensor_mul(w[:, 0:1], pr[:, 2:3], pr[:, 2:3])          # sum^2
        nc.vector.tensor_scalar_mul(w[:, 1:2], w[:, 0:1], 1.0 / n)
        nc.vector.tensor_sub(w[:, 2:3], pr[:, 3:4], w[:, 1:2])
        nc.vector.tensor_scalar_mul(w[:, 3:4], w[:, 2:3], 1.0 / (n - 1.0))  # var
        nc.scalar.activation(w[:, 4:5], w[:, 3:4], AF.Sqrt)              # std
        nc.vector.tensor_scalar(w[:, 5:6], w[:, 4:5], EPS, EPS * EPS,
                                ALU.mult, ALU.add)                       # c
        nc.vector.tensor_add(w[:, 6:7], w[:, 5:6], pr[:, 0:1])           # c - min
        nc.vector.tensor_add(w[:, 7:8], pr[:, 1:2], pr[:, 0:1])         # max - min
        nc.vector.tensor_add(r4[:, 0:1], w[:, 7:8], w[:, 5:6])          # + c
        nc.vector.reciprocal(r4[:, 1:2], r4[:, 0:1])                    # inv
        outsb = epil.tile([128, 512], dt, tag="outsb")
        nc.vector.tensor_scalar(outsb[:], rgbf[:], w[:, 6:7], r4[:, 1:2],
                                ALU.add, ALU.mult)
        nc.sync.dma_start(out_d[:].rearrange("(b p) y -> p b y", p=128),
                          outsb[:].rearrange("p (b y) -> p b y", b=2))
    return nc


# ----------------------------------------------------------------------------
# entry points
# ----------------------------------------------------------------------------

def _axis_aligned(R, T):
    return (np.allclose(np.asarray(R[0]), np.eye(3), atol=1e-6)
            and abs(float(T[0][0]) - float(T[0][1])) < 1e-12)


class _CachedSpmd:
    """Compile the PJRT executable once; repeat calls only transfer + exec."""

    def __init__(self, nc, n_cores):
        import jax
        from concourse import mybir
        from concourse.bass2jax import (_bass_exec_p, install_neuronx_cc_hook,
                                        partition_id_tensor)
        from jax.experimental.shard_map import shard_map
        from jax.sharding import Mesh, PartitionSpec
        install_neuronx_cc_hook()
        self.jax = jax
        self.n_cores = n_cores
        pname = nc.partition_id_tensor.name if nc.partition_id_tensor else None
        in_names, out_names, out_avals, zero_outs = [], [], [], []
        for alloc in nc.m.functions[0].allocations:
            if not isinstance(alloc, mybir.MemoryLocationSet):
                continue
            name = alloc.memorylocations[0].name
            if alloc.kind == "ExternalInput":
                if name != pname:
                    in_names.append(name)
            elif alloc.kind == "ExternalOutput":
                shape = tuple(alloc.tensor_shape)
                dtype = mybir.dt.np(alloc.dtype)
                out_names.append(name)
                out_avals.append(jax.core.ShapedArray(shape, dtype))
                zero_outs.append(np.zeros(shape, dtype))
        self.in_names, self.out_names = in_names, out_names
        self.out_avals, self.zero_outs = out_avals, zero_outs
        n_params, n_outs = len(in_names), len(out_names)
        all_in = list(in_names) + list(out_names)
        if pname is not None:
            all_in.append(pname)

        def _body(*args):
            operands = list(args)
            if pname is not None:
                operands.append(partition_id_tensor())
            outs = _bass_exec_p.bind(
                *operands, out_avals=tuple(out_avals), in_names=tuple(all_in),
                out_names=tuple(out_names), lowering_input_output_aliases=(),
                sim_require_finite=True, sim_require_nnan=True, nc=nc)
            return tuple(outs)

        devices = jax.devices()[:n_cores]
        mesh = Mesh(np.asarray(devices), ("core",))
        in_specs = (PartitionSpec("core"),) * (n_params + n_outs)
        out_specs = (PartitionSpec("core"),) * n_outs
        self.fn = jax.jit(shard_map(_body, mesh=mesh, in_specs=in_specs,
                                    out_specs=out_specs, check_rep=False),
                          keep_unused=True)
        self._dev_zeros = [jax.device_put(np.zeros(
            (n_cores * z.shape[0], *z.shape[1:]), z.dtype)) for z in zero_outs]

    def run(self, in_maps):
        jax = self.jax
        concat = [np.concatenate([np.asarray(in_maps[c][nm])
                                  for c in range(self.n_cores)], axis=0)
                  for nm in self.in_names]
        outs = self.fn(*concat, *self._dev_zeros)
        jax.block_until_ready(outs)
        return [{nm: np.asarray(outs[i]).reshape(
                    self.n_cores, *self.out_avals[i].shape)[c]
                 for i, nm in enumerate(self.out_names)}
                for c in range(self.n_cores)]


_RUNNER_CACHE = {}


def _run(image3d, R, T, trace=False, nrep=1):
    vol = np.ascontiguousarray(np.asarray(image3d, np.float32)[0, 0])
    in_maps, nd = _host_inputs(vol, np.asarray(T, np.float64)[0])
    for m in in_maps:
        m["nrep"] = np.full((1, 1), nrep, np.int32)
    if nd not in _NC_CACHE:
        nc = _build_nc(nd)
        nc.finalize()
        _NC_CACHE[nd] = nc
    nc = _NC_CACHE[nd]
    if id(nc) not in _RUNNER_CACHE:
        _RUNNER_CACHE[id(nc)] = _CachedSpmd(nc, N_CORES)
    results = _RUNNER_CACHE[id(nc)].run(in_maps)
    out = np.asarray(results[0]["out"], np.float32)[None, None]
    return out, results


def _numpy_fallback(image3d, R, T):
    """Direct port of the reference for non-axis-aligned cameras."""
    image3d = np.asarray(image3d, np.float32)
    R = np.asarray(R, np.float32); T = np.asarray(T, np.float32)
    B, C, D, H, W = image3d.shape
    vol = image3d[:, 0]
    vox = 3.0 / max(C, D)
    yg, xg = np.meshgrid(np.linspace(-1, 1, IMG), np.linspace(-1, 1, IMG),
                         indexing='ij')
    depths = np.linspace(MIN_D, MAX_D, NPTS)
    pcam = np.stack([xg[..., None] * depths / FOCAL,
                     yg[..., None] * depths / FOCAL,
                     np.broadcast_to(depths, (IMG, IMG, NPTS))], -1)
    v = pcam[None] - T[:, None, None, None, :]
    pw = np.einsum('bhwpj,bkj->bhwpk', v, R)
    half = np.array([vox * (W - 1) / 2, vox * (H - 1) / 2, vox * (D - 1) / 2])
    local = pw / half

    def tri(voln, pts):
        ix = (pts[..., 0] + 1) * .5 * (W - 1)
        iy = (pts[..., 1] + 1) * .5 * (H - 1)
        iz = (pts[..., 2] + 1) * .5 * (D - 1)
        out = np.zeros(ix.shape, np.float32)
        x0, y0, z0 = np.floor(ix), np.floor(iy), np.floor(iz)
        fx, fy, fz = ix - x0, iy - y0, iz - z0
        for zi, wz in ((z0, 1 - fz), (z0 + 1, fz)):
            for yi, wy in ((y0, 1 - fy), (y0 + 1, fy)):
                for xi, wx in ((x0, 1 - fx), (x0 + 1, fx)):
                    valid = ((xi >= 0) & (xi < W) & (yi >= 0) & (yi < H)
                             & (zi >= 0) & (zi < D))
                    vv = voln[np.clip(zi, 0, D - 1).astype(int),
                              np.clip(yi, 0, H - 1).astype(int),
                              np.clip(xi, 0, W - 1).astype(int)]
                    out += np.where(valid, vv * (wz * wy * wx), 0).astype(np.float32)
        return out

    feat = np.stack([tri(vol[b], local[b]) for b in range(B)])
    sigma = DENSITY * np.stack([tri(np.ones((D, H, W), np.float32), local[b])
                                for b in range(B)])
    t = (1.0 + 1e-10) - sigma
    ab = np.cumprod(t, -1)
    ab = np.concatenate([np.ones_like(ab[..., :1]), ab[..., :-1]], -1)
    rgb = np.sum(sigma * ab * feat, -1)
    out = np.transpose(rgb, (0, 2, 1))[:, None]
    s = (out - out.mean()) / (np.std(out, ddof=1) + EPS)
    return ((s - s.min() + EPS) / (s.max() - s.min() + EPS)).astype(np.float32)


def kernel(image3d, R, T):
    if not _axis_aligned(R, T):
        return _numpy_fallback(image3d, R, T)
    out, _ = _run(image3d, R, T, trace=False)
    return out



# revision 15
# speedup vs baseline: 7.7786x; 7.7786x over previous
"""Trainium2 Bass kernel for DirectVolumeRenderer (axis-aligned camera).

Factorization (per depth p, camera R=I so sample coords are separable):
    trilinear(vol) = z-lerp of 2 slices -> two matmuls with the SAME tent
    matrix  A_p[v,q] = relu(1 - |v - (a_p + s_p*q)|):
        T1   = Zp^T @ A_p          (contract y)
        feat = A_p^T @ T1          (contract x) -> image in [px,py] layout
    sigma_p = 0.1*az_p * av_p[px] (x) av_p[py]  (rank-1, host vectors)

Key simplification: transmittance Gamma_k is DATA-INDEPENDENT (density is
a constant 0.1 and the ray/volume geometry is fixed).  On sigma_k's
support (the nested valid square S_k) every earlier sigma_j was fully
inside its own square, so Gamma_k == gamma_k = prod_{j<k}(1 - 0.1*az_j),
a host-computable SCALAR (validated to ~3e-6 against the exact 2D
recurrence).  The device therefore computes only
    rgb = sum_k (gamma_k * sigma_k) .* feat_k
with gamma_k folded into the host-side sigma u-vectors -- no serial
compositing chain on the device at all.

Sharding: 240 active depths split into 8 contiguous runs of 30 per core.
Cross-core combine is one fp16 AllReduce(sum) + normalization.

Engines per depth: PE does sigma outer-product (f32r), mm1/mm2 (bf16) and
the rgb PSUM accumulation (bf16 identity matmul); ACT builds the tent and
the PSUM->SBUF T1 copy (scaled by -wz_large); DVE does the one-op z-lerp
(fp8 slices -> bf16) and the weight multiply.  Slab DMA is prefetched
in-loop (ring buffer) so compute chases the stream.
"""
import os
import sys
import numpy as np

for _p in ("/opt/trn_rl_repo", "/root/.axon_site/_ro/trn_rl_repo"):
    if os.path.isdir(_p) and _p not in sys.path:
        sys.path.insert(0, _p)

IMG = 256
NPTS = 320
MIN_D, MAX_D = 2.0, 6.0
FOCAL = 2.0
DENSITY = 0.1
EPS = 1e-8
N_CORES = 8


# ----------------------------------------------------------------------------
# host-side geometry
# ----------------------------------------------------------------------------

def _geometry(T):
    """Per-depth separable sampling params (f64). Requires R=I and Tx==Ty."""
    Tx, Ty, Tz = float(T[0]), float(T[1]), float(T[2])
    vox = 3.0 / 256.0
    half = vox * 255.0 * 0.5
    depths = np.linspace(MIN_D, MAX_D, NPTS)
    c = depths * 127.5 / (2.0 * half)
    s = c * (2.0 / 255.0)
    a = 127.5 - c - Tx * 127.5 / half
    iz = 127.5 * ((depths - Tz) / half + 1.0)
    z0 = np.floor(iz).astype(np.int64)
    fz = iz - z0
    z1 = z0 + 1
    wz0 = np.where((z0 >= 0) & (z0 < 256), 1.0 - fz, 0.0)
    wz1 = np.where((z1 >= 0) & (z1 < 256), fz, 0.0)
    az = wz0 + wz1
    q = np.arange(IMG)
    ic = a[:, None] + s[:, None] * q[None, :]
    c0 = np.floor(ic)
    fc = ic - c0
    av = (np.where((c0 >= 0) & (c0 < 256), 1.0 - fc, 0.0)
          + np.where((c0 + 1 >= 0) & (c0 + 1 < 256), fc, 0.0))
    return dict(s=s, a=a, z0=z0, z1=z1, wz0=wz0, wz1=wz1, az=az, av=av,
                active=az > 0)


def _host_inputs(vol, T):
    """Build the 8 per-core input maps. vol: (256,256,256) f32 (z,y,x)."""
    import ml_dtypes
    f8 = ml_dtypes.float8_e4m3
    g = _geometry(T)
    act = np.nonzero(g["active"])[0]
    nd = int(np.ceil(len(act) / N_CORES))

    # gamma_k = prod_{j<k} (1 - 0.1*az_j): global transmittance scalars
    cfac = 1.0 - DENSITY * g["az"]
    gam = np.ones(NPTS)
    gam[1:] = np.cumprod(cfac)[:-1]
    # fold gamma into the (negative) sigma u-vector
    uneg_all = (-DENSITY * (gam * g["az"])[:, None] * g["av"])
    v_all = g["av"]

    vol8 = vol.astype(f8)
    in_maps = []
    for cidx in range(N_CORES):
        ks = [int(act[i]) for i in range(cidx * nd, min((cidx + 1) * nd, len(act)))]

        slices = np.zeros((128, nd, 1024), f8)
        tb = np.zeros((128, 2 * nd), np.float32)
        tsc = np.zeros((128, nd), np.float32)
        rsc = np.zeros((128, nd), np.float32)
        wlp = np.zeros((128, nd), np.float32)
        vbs = np.zeros((128, nd, 512), ml_dtypes.bfloat16)
        prow = np.arange(128, dtype=np.float32)

        for j, p in enumerate(ks):
            w0, w1 = g["wz0"][p], g["wz1"][p]
            zz0 = min(max(int(g["z0"][p]), 0), 255)
            zz1 = min(max(int(g["z1"][p]), 0), 255)
            if w0 <= w1:
                z_small, z_large, w_small, w_large = zz0, zz1, w0, w1
            else:
                z_small, z_large, w_small, w_large = zz1, zz0, w1, w0
            # slot0 = small-weight slice, slot1 = large-weight slice
            for si, zz in ((0, z_small), (1, z_large)):
                sl = vol8[zz]                                # (y=256, x=256)
                slices[:, j, si * 512:(si + 1) * 512] = \
                    sl.reshape(2, 128, 256).transpose(1, 0, 2).reshape(128, 512)
            rsc[:, j] = np.float32(w_small / w_large)
            wlp[:, j] = np.float32(-w_large)
            tsc[:, j] = np.float32(-g["s"][p])
            for b in (0, 1):
                tb[:, 2 * j + b] = (b * 128 + prow) - np.float32(g["a"][p])
                # sigma field (gamma folded, negative): vb[p,256b+py]
                vbs[:, j, 256 * b:256 * (b + 1)] = np.outer(
                    uneg_all[p][128 * b:128 * (b + 1)], v_all[p])

        pyio = np.broadcast_to(np.arange(256, dtype=np.float32), (128, 256)).copy()
        in_maps.append({
            "slices": slices.reshape(128, nd * 1024),
            "tb": tb, "tsc": tsc, "rsc": rsc, "wlp": wlp,
            "vbs": vbs.reshape(128, nd * 512), "pyio": pyio,
            "identh": np.eye(128, dtype=ml_dtypes.bfloat16),
            "identf": np.eye(128, dtype=np.float32),
            "ones1": np.ones((1, 128), np.float32),
            "onesc": np.ones((128, 1), np.float32),
        })
    return in_maps, nd


# ----------------------------------------------------------------------------
# device program
# ----------------------------------------------------------------------------

_NC_CACHE = {}


def _build_nc(nd, sim=False):
    """sim=True replaces the AllReduce with a local DMA copy so the
    single-core TimelineSim cost model can run the program."""
    import concourse.bass as bass
    import concourse.tile as tile
    from concourse import bacc, mybir
    from contextlib import ExitStack

    dt = mybir.dt.float32
    dr = mybir.dt.float32r
    dh = mybir.dt.bfloat16
    d8 = mybir.dt.float8e4
    dhalf = mybir.dt.float16
    AF = mybir.ActivationFunctionType
    ALU = mybir.AluOpType
    AX = mybir.AxisListType.X

    nc = bacc.Bacc(None, num_devices=N_CORES)
    slices = nc.dram_tensor("slices", [128, nd * 1024], d8, kind="ExternalInput")
    tb_d = nc.dram_tensor("tb", [128, 2 * nd], dt, kind="ExternalInput")
    tsc_d = nc.dram_tensor("tsc", [128, nd], dt, kind="ExternalInput")
    rsc_d = nc.dram_tensor("rsc", [128, nd], dt, kind="ExternalInput")
    wlp_d = nc.dram_tensor("wlp", [128, nd], dt, kind="ExternalInput")
    vbs_d = nc.dram_tensor("vbs", [128, nd * 512], dh, kind="ExternalInput")
    pyio_d = nc.dram_tensor("pyio", [128, 256], dt, kind="ExternalInput")
    idh_d = nc.dram_tensor("identh", [128, 128], dh, kind="ExternalInput")
    idf_d = nc.dram_tensor("identf", [128, 128], dt, kind="ExternalInput")
    ones1_d = nc.dram_tensor("ones1", [1, 128], dt, kind="ExternalInput")
    onesc_d = nc.dram_tensor("onesc", [128, 1], dt, kind="ExternalInput")
    out_d = nc.dram_tensor("out", [256, 256], dt, kind="ExternalOutput")
    cc_in = nc.dram_tensor("cc_in", [256, 256], dhalf)
    cc_out = nc.dram_tensor("cc_out", [256, 256], dhalf, addr_space="Shared")

    with tile.TileContext(nc) as tc, ExitStack() as ctx:
        const = ctx.enter_context(tc.tile_pool(name="const", bufs=1))
        slp = ctx.enter_context(tc.tile_pool(name="slp", bufs=4))
        work = ctx.enter_context(tc.tile_pool(name="work", bufs=3))
        epil = ctx.enter_context(tc.tile_pool(name="epil", bufs=1))
        psum = ctx.enter_context(
            tc.tile_pool(name="psum", bufs=2, space=bass.MemorySpace.PSUM))
        pst1 = ctx.enter_context(
            tc.tile_pool(name="pst1", bufs=3, space=bass.MemorySpace.PSUM))
        psacc = ctx.enter_context(
            tc.tile_pool(name="psacc", bufs=1, space=bass.MemorySpace.PSUM))

        def cload(dram, shape, dtype=dt):
            t = const.tile(shape, dtype, tag=dram.name)
            nc.sync.dma_start(t[:], dram[:])
            return t

        tb = cload(tb_d, [128, 2 * nd])
        tsc = cload(tsc_d, [128, nd])
        rsc = cload(rsc_d, [128, nd])
        wlp = cload(wlp_d, [128, nd])
        pyio = cload(pyio_d, [128, 256])
        identh = cload(idh_d, [128, 128], dh)
        identf = cload(idf_d, [128, 128], dt)
        ones1 = cload(ones1_d, [1, 128], dt)
        onesc = cload(onesc_d, [128, 1], dt)

        NCH = (nd + 1) // 2
        PREF = 3
        slabs = [None] * NCH
        vbsl = [None] * NCH

        def issue_chunk(j):
            ndep = min(2, nd - 2 * j)
            t = slp.tile([128, min(2048, ndep * 1024)], d8, tag="slab")
            nc.sync.dma_start(t[:], slices[:, j * 2048:j * 2048 + t.shape[1]])
            slabs[j] = t
            v = slp.tile([128, ndep * 512], dh, tag="vbs")
            nc.sync.dma_start(v[:], vbs_d[:, j * 1024:j * 1024 + v.shape[1]])
            vbsl[j] = v

        for j in range(min(PREF, NCH)):
            issue_chunk(j)

        rgbps = psacc.tile([128, 512], dt, tag="rgb")

        # software-pipelined state
        at_t = [None] * nd      # tent SBUF tiles
        zm_t = [None] * nd      # z-merged slice tiles
        wf_t = [None] * nd      # weighted feature tiles

        def emit_tent(k):
            dab = work.tile([128, 512], dh, tag="dab")
            for b in (0, 1):
                nc.scalar.activation(dab[:, 256 * b:256 * (b + 1)], pyio[:],
                                     AF.Abs, bias=tb[:, 2 * k + b:2 * k + b + 1],
                                     scale=tsc[:, k:k + 1])
            at = work.tile([128, 512], dh, tag="at")
            nc.scalar.activation(at[:], dab[:], AF.Relu, bias=1.0, scale=-1.0)
            at_t[k] = at

        def emit_zm(k):
            j = k // 2
            base = (k % 2) * 1024
            zm = work.tile([128, 512], dh, tag="zm")
            nc.vector.scalar_tensor_tensor(
                zm[:], slabs[j][:, base:base + 512], rsc[:, k:k + 1],
                slabs[j][:, base + 512:base + 1024], ALU.mult, ALU.add)
            zm_t[k] = zm

        # prologue for depth 0
        emit_zm(0)
        emit_tent(0)

        for k in range(nd):
            zm = zm_t[k]
            at = at_t[k]

            # prefetch the slab chunk PREF ahead (once per chunk)
            if k % 2 == 0 and k // 2 + PREF < NCH:
                issue_chunk(k // 2 + PREF)

            # --- mm1: T1[x,py] = sum_y Zp[y,x] * A[y,py] ---
            t1ps = pst1.tile([128, 512], dt, tag="t1")
            for xc in (0, 1):
                for yb in (0, 1):
                    nc.tensor.matmul(
                        t1ps[:, 256 * xc:256 * (xc + 1)],
                        zm[:, 256 * yb + 128 * xc:256 * yb + 128 * xc + 128],
                        at[:, 256 * yb:256 * (yb + 1)],
                        start=(yb == 0), stop=(yb == 1))

            # PE filler while ACT does the t1 copy: prev depth's rgb acc
            if k > 0:
                nc.tensor.matmul(rgbps[:], identh[:], wf_t[k - 1][:],
                                 start=(k == 1), stop=False, skip_group_check=True)

            # --- ACT: t1sb = -wz_large * T1  (PSUM->SBUF, bf16) ---
            t1sb = work.tile([128, 512], dh, tag="t1sb")
            nc.scalar.activation(t1sb[:], t1ps[:], AF.Copy, scale=wlp[:, k:k + 1])
            if k + 1 < nd:
                emit_tent(k + 1)

            # --- DVE: z-merge for next depth ---
            if k + 1 < nd:
                emit_zm(k + 1)

            # --- mm2: -feat[px,py] = sum_x A[x,px] * t1sb[x,py] ---
            featps = psum.tile([128, 512], dt, tag="feat")
            for mb in (0, 1):
                for xb in (0, 1):
                    nc.tensor.matmul(
                        featps[:, 256 * mb:256 * (mb + 1)],
                        at[:, 256 * xb + 128 * mb:256 * xb + 128 * mb + 128],
                        t1sb[:, 256 * xb:256 * (xb + 1)],
                        start=(xb == 0), stop=(xb == 1))

            # --- DVE: wf = (-gamma*sigma) .* (-feat) = gamma*sigma*feat ---
            j = k // 2
            vbk = vbsl[j][:, (k % 2) * 512:(k % 2) * 512 + 512]
            wf = work.tile([128, 512], dh, tag="wf")
            nc.vector.tensor_mul(wf[:], vbk, featps[:])
            wf_t[k] = wf

        nc.tensor.matmul(rgbps[:], identh[:], wf_t[nd - 1][:],
                         start=False, stop=True, skip_group_check=True)

        # ---- cross-core reduce (fp16 AllReduce) ----
        rgbh = epil.tile([128, 512], dhalf, tag="rgbh")
        nc.vector.tensor_copy(rgbh[:], rgbps[:])
        nc.sync.dma_start(cc_in[:].rearrange("(b p) y -> p b y", p=128),
                          rgbh[:].rearrange("p (b y) -> p b y", b=2))
        if sim:
            nc.sync.dma_start(cc_out[:], cc_in[:])
        else:
            nc.gpsimd.collective_compute(
                "AllReduce", ALU.add, replica_groups=[list(range(N_CORES))],
                ins=[cc_in[:]], outs=[cc_out[:]])
        rgbfh = epil.tile([128, 512], dhalf, tag="rgbfh")
        nc.sync.dma_start(rgbfh[:].rearrange("p (b y) -> p b y", b=2),
                          cc_out[:].rearrange("(b p) y -> p b y", p=128))
        rgbf = epil.tile([128, 512], dt, tag="rgbf")
        nc.vector.tensor_copy(rgbf[:], rgbfh[:])

        # ---- normalization: global min/max/sum/sumsq then affine ----
        sq = epil.tile([128, 512], dt, tag="sq")
        nc.vector.tensor_mul(sq[:], rgbf[:], rgbf[:])
        r4 = epil.tile([128, 4], dt, tag="r4")
        nc.vector.tensor_reduce(r4[:, 0:1], rgbf[:], axis=AX, op=ALU.min)
        nc.vector.tensor_reduce(r4[:, 1:2], rgbf[:], axis=AX, op=ALU.max)
        nc.vector.tensor_reduce(r4[:, 2:3], rgbf[:], axis=AX, op=ALU.add)
        nc.vector.tensor_reduce(r4[:, 3:4], sq[:], axis=AX, op=ALU.add)
        nc.vector.tensor_scalar_mul(r4[:, 1:2], r4[:, 1:2], -1.0)   # -max
        # transpose [min,-max] rows to partitions 0:2 via matmul, then a
        # single MIN reduce over the free dim gives [min, -max] per row
        trt = pst1.tile([128, 512], dt, tag="t1")
        tr = trt[0:2, 0:128]
        nc.tensor.matmul(tr, r4[:, 0:2], identf[:], start=True, stop=True)
        s2 = epil.tile([2, 1], dt, tag="s2")
        nc.vector.tensor_reduce(s2[0:2, 0:1], tr, axis=AX, op=ALU.min)
        # gather stats into one partition: gm = [min, -max, sum, sumsq]
        g4t = pst1.tile([128, 512], dt, tag="t1")
        g4 = g4t[0:1, 0:4]
        nc.tensor.matmul(g4[0:1, 0:2], s2[0:2, 0:1], identf[0:2, 0:2],
                         start=True, stop=True, skip_group_check=True)
        nc.tensor.matmul(g4[0:1, 2:4], onesc[:, 0:1], r4[:, 2:4],
                         start=True, stop=True, skip_group_check=True)
        gs = epil.tile([1, 4], dt, tag="gs")
        nc.scalar.activation(gs[:], g4, AF.Copy)
        # scalar chain on partition 0:  (see derivation in baseline)
        n = float(IMG * IMG)
        w = epil.tile([1, 8], dt, tag="w")
        nc.vector.tensor_mul(w[:, 0:1], gs[:, 2:3], gs[:, 2:3])      # sum^2
        nc.vector.tensor_scalar_mul(w[:, 1:2], w[:, 0:1], 1.0 / n)
        nc.vector.tensor_sub(w[:, 2:3], gs[:, 3:4], w[:, 1:2])
        nc.vector.tensor_scalar_mul(w[:, 3:4], w[:, 2:3], 1.0 / (n - 1.0))  # var
        nc.scalar.activation(w[:, 4:5], w[:, 3:4], AF.Sqrt)              # std
        nc.vector.tensor_scalar(w[:, 5:6], w[:, 4:5], EPS, EPS * EPS,
                                ALU.mult, ALU.add)                       # c
        sc2 = epil.tile([1, 2], dt, tag="sc2")
        nc.vector.tensor_sub(sc2[:, 0:1], w[:, 5:6], gs[:, 0:1])      # c - min
        nc.vector.tensor_add(w[:, 6:7], gs[:, 0:1], gs[:, 1:2])     # min-max
        nc.vector.tensor_sub(w[:, 7:8], w[:, 5:6], w[:, 6:7])           # c+max-min
        nc.vector.reciprocal(sc2[:, 1:2], w[:, 7:8])                    # inv
        # broadcast [1,2] -> [128,2]
        bct = psum.tile([128, 512], dt, tag="vb")
        bc = bct[:, 0:2]
        nc.tensor.matmul(bc, ones1[:], sc2[:], start=True, stop=True)
        bcs = epil.tile([128, 2], dt, tag="bcs")
        nc.scalar.activation(bcs[:], bc, AF.Copy)
        outsb = epil.tile([128, 512], dt, tag="outsb")
        nc.vector.tensor_scalar(outsb[:], rgbf[:], bcs[:, 0:1], bcs[:, 1:2],
                                ALU.add, ALU.mult)
        nc.sync.dma_start(out_d[:].rearrange("(b p) y -> p b y", p=128),
                          outsb[:].rearrange("p (b y) -> p b y", b=2))
    return nc


# ----------------------------------------------------------------------------
# entry points
# ----------------------------------------------------------------------------

def _axis_aligned(R, T):
    return (np.allclose(np.asarray(R[0]), np.eye(3), atol=1e-6)
            and abs(float(T[0][0]) - float(T[0][1])) < 1e-12)


class _CachedSpmd:
    """Compile the PJRT executable once; repeat calls only transfer + exec."""

    def __init__(self, nc, n_cores):
        import jax
        from concourse import mybir
        from concourse.bass2jax import (_bass_exec_p, install_neuronx_cc_hook,
                                        partition_id_tensor)
        from jax.experimental.shard_map import shard_map
        from jax.sharding import Mesh, PartitionSpec
        install_neuronx_cc_hook()
        self.jax = jax
        self.n_cores = n_cores
        pname = nc.partition_id_tensor.name if nc.partition_id_tensor else None
        in_names, out_names, out_avals, zero_outs = [], [], [], []
        for alloc in nc.m.functions[0].allocations:
            if not isinstance(alloc, mybir.MemoryLocationSet):
                continue
            name = alloc.memorylocations[0].name
            if alloc.kind == "ExternalInput":
                if name != pname:
                    in_names.append(name)
            elif alloc.kind == "ExternalOutput":
                shape = tuple(alloc.tensor_shape)
                dtype = mybir.dt.np(alloc.dtype)
                out_names.append(name)
                out_avals.append(jax.core.ShapedArray(shape, dtype))
                zero_outs.append(np.zeros(shape, dtype))
        self.in_names, self.out_names = in_names, out_names
        self.out_avals, self.zero_outs = out_avals, zero_outs
        n_params, n_outs = len(in_names), len(out_names)
        all_in = list(in_names) + list(out_names)
        if pname is not None:
            all_in.append(pname)

        def _body(*args):
            operands = list(args)
            if pname is not None:
                operands.append(partition_id_tensor())
            outs = _bass_exec_p.bind(
                *operands, out_avals=tuple(out_avals), in_names=tuple(all_in),
                out_names=tuple(out_names), lowering_input_output_aliases=(),
                sim_require_finite=True, sim_require_nnan=True, nc=nc)
            return tuple(outs)

        devices = jax.devices()[:n_cores]
        mesh = Mesh(np.asarray(devices), ("core",))
        in_specs = (PartitionSpec("core"),) * (n_params + n_outs)
        out_specs = (PartitionSpec("core"),) * n_outs
        self.fn = jax.jit(shard_map(_body, mesh=mesh, in_specs=in_specs,
                                    out_specs=out_specs, check_rep=False),
                          keep_unused=True)
        self._dev_zeros = [jax.device_put(np.zeros(
            (n_cores * z.shape[0], *z.shape[1:]), z.dtype)) for z in zero_outs]

    def run(self, in_maps):
        jax = self.jax
        concat = [np.concatenate([np.asarray(in_maps[c][nm])
                                  for c in range(self.n_cores)], axis=0)
                  for nm in self.in_names]
        outs = self.fn(*concat, *self._dev_zeros)
        jax.block_until_ready(outs)
        return [{nm: np.asarray(outs[i]).reshape(
                    self.n_cores, *self.out_avals[i].shape)[c]
                 for i, nm in enumerate(self.out_names)}
                for c in range(self.n_cores)]


_RUNNER_CACHE = {}


def _run(image3d, R, T, trace=False):
    vol = np.ascontiguousarray(np.asarray(image3d, np.float32)[0, 0])
    in_maps, nd = _host_inputs(vol, np.asarray(T, np.float64)[0])
    if nd not in _NC_CACHE:
        nc = _build_nc(nd)
        nc.finalize()
        _NC_CACHE[nd] = nc
    nc = _NC_CACHE[nd]
    if id(nc) not in _RUNNER_CACHE:
        _RUNNER_CACHE[id(nc)] = _CachedSpmd(nc, N_CORES)
    results = _RUNNER_CACHE[id(nc)].run(in_maps)
    out = np.asarray(results[0]["out"], np.float32)[None, None]
    return out, results


def _numpy_fallback(image3d, R, T):
    """Direct port of the reference for non-axis-aligned cameras."""
    image3d = np.asarray(image3d, np.float32)
    R = np.asarray(R, np.float32); T = np.asarray(T, np.float32)
    B, C, D, H, W = image3d.shape
    vol = image3d[:, 0]
    vox = 3.0 / max(C, D)
    yg, xg = np.meshgrid(np.linspace(-1, 1, IMG), np.linspace(-1, 1, IMG),
                         indexing='ij')
    depths = np.linspace(MIN_D, MAX_D, NPTS)
    pcam = np.stack([xg[..., None] * depths / FOCAL,
                     yg[..., None] * depths / FOCAL,
                     np.broadcast_to(depths, (IMG, IMG, NPTS))], -1)
    v = pcam[None] - T[:, None, None, None, :]
    pw = np.einsum('bhwpj,bkj->bhwpk', v, R)
    half = np.array([vox * (W - 1) / 2, vox * (H - 1) / 2, vox * (D - 1) / 2])
    local = pw / half

    def tri(voln, pts):
        ix = (pts[..., 0] + 1) * .5 * (W - 1)
        iy = (pts[..., 1] + 1) * .5 * (H - 1)
        iz = (pts[..., 2] + 1) * .5 * (D - 1)
        out = np.zeros(ix.shape, np.float32)
        x0, y0, z0 = np.floor(ix), np.floor(iy), np.floor(iz)
        fx, fy, fz = ix - x0, iy - y0, iz - z0
        for zi, wz in ((z0, 1 - fz), (z0 + 1, fz)):
            for yi, wy in ((y0, 1 - fy), (y0 + 1, fy)):
                for xi, wx in ((x0, 1 - fx), (x0 + 1, fx)):
                    valid = ((xi >= 0) & (xi < W) & (yi >= 0) & (yi < H)
                             & (zi >= 0) & (zi < D))
                    vv = voln[np.clip(zi, 0, D - 1).astype(int),
                              np.clip(yi, 0, H - 1).astype(int),
                              np.clip(xi, 0, W - 1).astype(int)]
                    out += np.where(valid, vv * (wz * wy * wx), 0).astype(np.float32)
        return out

    feat = np.stack([tri(vol[b], local[b]) for b in range(B)])
    sigma = DENSITY * np.stack([tri(np.ones((D, H, W), np.float32), local[b])
                                for b in range(B)])
    t = (1.0 + 1e-10) - sigma
    ab = np.cumprod(t, -1)
    ab = np.concatenate([np.ones_like(ab[..., :1]), ab[..., :-1]], -1)
    rgb = np.sum(sigma * ab * feat, -1)
    out = np.transpose(rgb, (0, 2, 1))[:, None]
    s = (out - out.mean()) / (np.std(out, ddof=1) + EPS)
    return ((s - s.min() + EPS) / (s.max() - s.min() + EPS)).astype(np.float32)


def kernel(image3d, R, T):
    if not _axis_aligned(R, T):
        return _numpy_fallback(image3d, R, T)
    out, _ = _run(image3d, R, T, trace=False)
    return out


# revision 18
# speedup vs baseline: 12.3730x; 1.5907x over previous
"""Trainium2 Bass kernel for DirectVolumeRenderer (axis-aligned camera).

Factorization (per depth p, camera R=I so sample coords are separable):
    trilinear(vol) = z-lerp of 2 slices -> two matmuls with the SAME tent
    matrix  A_p[v,q] = relu(1 - |v - (a_p + s_p*q)|):
        T1   = Zp^T @ A_p          (contract y)
        feat = A_p^T @ T1          (contract x) -> image in [px,py] layout
    sigma_p = 0.1*az_p * av_p[px] (x) av_p[py]  (rank-1, host vectors)

Key simplification: transmittance Gamma_k is DATA-INDEPENDENT (density is
a constant 0.1 and the ray/volume geometry is fixed).  On sigma_k's
support (the nested valid square S_k) every earlier sigma_j was fully
inside its own square, so Gamma_k == gamma_k = prod_{j<k}(1 - 0.1*az_j),
a host-computable SCALAR (validated to ~3e-6 against the exact 2D
recurrence).  The device therefore computes only
    rgb = sum_k (gamma_k * sigma_k) .* feat_k
with gamma_k folded into the host-side sigma u-vectors -- no serial
compositing chain on the device at all.

Sharding: 240 active depths split into 8 contiguous runs of 30 per core.
Cross-core combine is one fp16 AllReduce(sum) + normalization.

Engines per depth: PE does sigma outer-product (f32r), mm1/mm2 (bf16) and
the rgb PSUM accumulation (bf16 identity matmul); ACT builds the tent and
the PSUM->SBUF T1 copy (scaled by -wz_large); DVE does the one-op z-lerp
(fp8 slices -> bf16) and the weight multiply.  Slab DMA is prefetched
in-loop (ring buffer) so compute chases the stream.
"""
import os
import sys
import numpy as np

for _p in ("/opt/trn_rl_repo", "/root/.axon_site/_ro/trn_rl_repo"):
    if os.path.isdir(_p) and _p not in sys.path:
        sys.path.insert(0, _p)

IMG = 256
NPTS = 320
MIN_D, MAX_D = 2.0, 6.0
FOCAL = 2.0
DENSITY = 0.1
EPS = 1e-8
N_CORES = 8


# ----------------------------------------------------------------------------
# host-side geometry
# ----------------------------------------------------------------------------

def _geometry(T):
    """Per-depth separable sampling params (f64). Requires R=I and Tx==Ty."""
    Tx, Ty, Tz = float(T[0]), float(T[1]), float(T[2])
    vox = 3.0 / 256.0
    half = vox * 255.0 * 0.5
    depths = np.linspace(MIN_D, MAX_D, NPTS)
    c = depths * 127.5 / (2.0 * half)
    s = c * (2.0 / 255.0)
    a = 127.5 - c - Tx * 127.5 / half
    iz = 127.5 * ((depths - Tz) / half + 1.0)
    z0 = np.floor(iz).astype(np.int64)
    fz = iz - z0
    z1 = z0 + 1
    wz0 = np.where((z0 >= 0) & (z0 < 256), 1.0 - fz, 0.0)
    wz1 = np.where((z1 >= 0) & (z1 < 256), fz, 0.0)
    az = wz0 + wz1
    q = np.arange(IMG)
    ic = a[:, None] + s[:, None] * q[None, :]
    c0 = np.floor(ic)
    fc = ic - c0
    av = (np.where((c0 >= 0) & (c0 < 256), 1.0 - fc, 0.0)
          + np.where((c0 + 1 >= 0) & (c0 + 1 < 256), fc, 0.0))
    return dict(s=s, a=a, z0=z0, z1=z1, wz0=wz0, wz1=wz1, az=az, av=av,
                active=az > 0)


def _host_inputs(vol, T):
    """Build the 8 per-core input maps. vol: (256,256,256) f32 (z,y,x)."""
    import ml_dtypes
    g = _geometry(T)
    act = np.nonzero(g["active"])[0]

    # gamma_k = prod_{j<k} (1 - 0.1*az_j): global transmittance scalars
    cfac = 1.0 - DENSITY * g["az"]
    gam = np.ones(NPTS)
    gam[1:] = np.cumprod(cfac)[:-1]
    # truncate depths whose remaining transmittance is negligible
    # (gamma < 4e-5 -> contribution ~1e-4 of the image; validated 5.7e-5
    # rel err at 96 of 240 depths)
    act = np.array([p for p in act if gam[p] > 4e-5])
    nd = int(np.ceil(len(act) / N_CORES))
    # fold gamma into the (negative) sigma u-vector
    uneg_all = (-DENSITY * (gam * g["az"])[:, None] * g["av"])
    v_all = g["av"]

    vol16 = vol.astype(ml_dtypes.bfloat16)
    in_maps = []
    for cidx in range(N_CORES):
        ks = [int(act[i]) for i in range(cidx * nd, min((cidx + 1) * nd, len(act)))]

        slices = np.zeros((128, nd, 1024), ml_dtypes.bfloat16)
        tb = np.zeros((128, 2 * nd), np.float32)
        tsc = np.zeros((128, nd), np.float32)
        rsc = None
        wlp = np.zeros((128, nd), np.float32)
        vbs = np.zeros((128, nd, 512), ml_dtypes.bfloat16)
        prow = np.arange(128, dtype=np.float32)

        for j, p in enumerate(ks):
            w0, w1 = g["wz0"][p], g["wz1"][p]
            zz0 = min(max(int(g["z0"][p]), 0), 255)
            zz1 = min(max(int(g["z1"][p]), 0), 255)
            if w0 <= w1:
                z_small, z_large, w_small, w_large = zz0, zz1, w0, w1
            else:
                z_small, z_large, w_small, w_large = zz1, zz0, w1, w0
            # slot0 = (w_small/w_large)-prescaled small slice, slot1 = large
            r = np.float32(w_small / w_large)
            for si, zz, sc in ((0, z_small, r), (1, z_large, np.float32(1.0))):
                sl = (vol16[zz].astype(np.float32) * sc).astype(vol16.dtype)
                slices[:, j, si * 512:(si + 1) * 512] = \
                    sl.reshape(2, 128, 256).transpose(1, 0, 2).reshape(128, 512)
            wlp[:, j] = np.float32(-w_large)
            tsc[:, j] = np.float32(-g["s"][p])
            for b in (0, 1):
                tb[:, 2 * j + b] = (b * 128 + prow) - np.float32(g["a"][p])
                # sigma field (gamma folded, negative): vb[p,256b+py]
                vbs[:, j, 256 * b:256 * (b + 1)] = np.outer(
                    uneg_all[p][128 * b:128 * (b + 1)], v_all[p])

        pyio = np.broadcast_to(np.arange(256, dtype=np.float32), (128, 256)).copy()
        in_maps.append({
            "slices": slices.reshape(128, nd * 1024),
            "tb": tb, "tsc": tsc, "wlp": wlp,
            "vbs": vbs.reshape(128, nd * 512), "pyio": pyio,
            "identh": np.eye(128, dtype=ml_dtypes.bfloat16),
            "identf": np.eye(128, dtype=np.float32),
            "ones1": np.ones((1, 128), np.float32),
            "onesc": np.ones((128, 1), np.float32),
        })
    return in_maps, nd


# ----------------------------------------------------------------------------
# device program
# ----------------------------------------------------------------------------

_NC_CACHE = {}


def _build_nc(nd, sim=False):
    """sim=True replaces the AllReduce with a local DMA copy so the
    single-core TimelineSim cost model can run the program."""
    import concourse.bass as bass
    import concourse.tile as tile
    from concourse import bacc, mybir
    from contextlib import ExitStack

    dt = mybir.dt.float32
    dr = mybir.dt.float32r
    dh = mybir.dt.bfloat16
    d8 = mybir.dt.float8e4
    dhalf = mybir.dt.float16
    AF = mybir.ActivationFunctionType
    ALU = mybir.AluOpType
    AX = mybir.AxisListType.X

    nc = bacc.Bacc(None, num_devices=N_CORES)
    slices = nc.dram_tensor("slices", [128, nd * 1024], dh, kind="ExternalInput")
    tb_d = nc.dram_tensor("tb", [128, 2 * nd], dt, kind="ExternalInput")
    tsc_d = nc.dram_tensor("tsc", [128, nd], dt, kind="ExternalInput")
    wlp_d = nc.dram_tensor("wlp", [128, nd], dt, kind="ExternalInput")
    vbs_d = nc.dram_tensor("vbs", [128, nd * 512], dh, kind="ExternalInput")
    pyio_d = nc.dram_tensor("pyio", [128, 256], dt, kind="ExternalInput")
    idh_d = nc.dram_tensor("identh", [128, 128], dh, kind="ExternalInput")
    idf_d = nc.dram_tensor("identf", [128, 128], dt, kind="ExternalInput")
    ones1_d = nc.dram_tensor("ones1", [1, 128], dt, kind="ExternalInput")
    onesc_d = nc.dram_tensor("onesc", [128, 1], dt, kind="ExternalInput")
    out_d = nc.dram_tensor("out", [256, 256], dt, kind="ExternalOutput")
    cc_in = nc.dram_tensor("cc_in", [256, 256], dhalf)
    cc_out = nc.dram_tensor("cc_out", [256, 256], dhalf, addr_space="Shared")

    with tile.TileContext(nc) as tc, ExitStack() as ctx:
        const = ctx.enter_context(tc.tile_pool(name="const", bufs=1))
        slp = ctx.enter_context(tc.tile_pool(name="slp", bufs=4))
        work = ctx.enter_context(tc.tile_pool(name="work", bufs=3))
        epil = ctx.enter_context(tc.tile_pool(name="epil", bufs=1))
        psum = ctx.enter_context(
            tc.tile_pool(name="psum", bufs=2, space=bass.MemorySpace.PSUM))
        pst1 = ctx.enter_context(
            tc.tile_pool(name="pst1", bufs=3, space=bass.MemorySpace.PSUM))
        psacc = ctx.enter_context(
            tc.tile_pool(name="psacc", bufs=1, space=bass.MemorySpace.PSUM))

        def cload(dram, shape, dtype=dt):
            t = const.tile(shape, dtype, tag=dram.name)
            nc.sync.dma_start(t[:], dram[:])
            return t

        tb = cload(tb_d, [128, 2 * nd])
        tsc = cload(tsc_d, [128, nd])
        wlp = cload(wlp_d, [128, nd])
        pyio = cload(pyio_d, [128, 256])
        identh = cload(idh_d, [128, 128], dh)
        identf = cload(idf_d, [128, 128], dt)
        ones1 = cload(ones1_d, [1, 128], dt)
        onesc = cload(onesc_d, [128, 1], dt)

        NCH = (nd + 1) // 2
        PREF = 3
        slabs = [None] * NCH
        vbsl = [None] * NCH

        def issue_chunk(j):
            ndep = min(2, nd - 2 * j)
            t = slp.tile([128, min(2048, ndep * 1024)], dh, tag="slab")
            nc.sync.dma_start(t[:], slices[:, j * 2048:j * 2048 + t.shape[1]])
            slabs[j] = t
            v = slp.tile([128, ndep * 512], dh, tag="vbs")
            nc.sync.dma_start(v[:], vbs_d[:, j * 1024:j * 1024 + v.shape[1]])
            vbsl[j] = v

        for j in range(min(PREF, NCH)):
            issue_chunk(j)

        rgbps = psacc.tile([128, 512], dt, tag="rgb")

        # software-pipelined state
        at_t = [None] * nd      # tent SBUF tiles
        zm_t = [None] * nd      # z-merged slice tiles
        wf_t = [None] * nd      # weighted feature tiles

        def emit_tent(k):
            dab = work.tile([128, 512], dh, tag="dab")
            for b in (0, 1):
                nc.scalar.activation(dab[:, 256 * b:256 * (b + 1)], pyio[:],
                                     AF.Abs, bias=tb[:, 2 * k + b:2 * k + b + 1],
                                     scale=tsc[:, k:k + 1])
            at = work.tile([128, 512], dh, tag="at")
            nc.scalar.activation(at[:], dab[:], AF.Relu, bias=1.0, scale=-1.0)
            at_t[k] = at

        def emit_zm(k):
            j = k // 2
            base = (k % 2) * 1024
            zm = work.tile([128, 512], dh, tag="zm")
            nc.gpsimd.tensor_add(zm[:], slabs[j][:, base:base + 512],
                                 slabs[j][:, base + 512:base + 1024])
            zm_t[k] = zm

        # prologue for depth 0
        emit_zm(0)
        emit_tent(0)

        for k in range(nd):
            zm = zm_t[k]
            at = at_t[k]

            # prefetch the slab chunk PREF ahead (once per chunk)
            if k % 2 == 0 and k // 2 + PREF < NCH:
                issue_chunk(k // 2 + PREF)

            # --- mm1: T1[x,py] = sum_y Zp[y,x] * A[y,py] ---
            t1ps = pst1.tile([128, 512], dt, tag="t1")
            for xc in (0, 1):
                for yb in (0, 1):
                    nc.tensor.matmul(
                        t1ps[:, 256 * xc:256 * (xc + 1)],
                        zm[:, 256 * yb + 128 * xc:256 * yb + 128 * xc + 128],
                        at[:, 256 * yb:256 * (yb + 1)],
                        start=(yb == 0), stop=(yb == 1))

            # PE filler while ACT does the t1 copy: prev depth's rgb acc
            if k > 0:
                nc.tensor.matmul(rgbps[:], identh[:], wf_t[k - 1][:],
                                 start=(k == 1), stop=False, skip_group_check=True)

            # --- DVE: t1sb = -wz_large * T1  (PSUM->SBUF, bf16) ---
            t1sb = work.tile([128, 512], dh, tag="t1sb")
            nc.vector.tensor_scalar_mul(t1sb[:], t1ps[:], wlp[:, k:k + 1])
            if k + 1 < nd:
                emit_tent(k + 1)

            # --- DVE: z-merge for next depth ---
            if k + 1 < nd:
                emit_zm(k + 1)

            # --- mm2: -feat[px,py] = sum_x A[x,px] * t1sb[x,py] ---
            featps = psum.tile([128, 512], dt, tag="feat")
            for mb in (0, 1):
                for xb in (0, 1):
                    nc.tensor.matmul(
                        featps[:, 256 * mb:256 * (mb + 1)],
                        at[:, 256 * xb + 128 * mb:256 * xb + 128 * mb + 128],
                        t1sb[:, 256 * xb:256 * (xb + 1)],
                        start=(xb == 0), stop=(xb == 1))

            # --- DVE: wf = (-gamma*sigma) .* (-feat) = gamma*sigma*feat ---
            j = k // 2
            vbk = vbsl[j][:, (k % 2) * 512:(k % 2) * 512 + 512]
            wf = work.tile([128, 512], dh, tag="wf")
            nc.vector.tensor_mul(wf[:], vbk, featps[:])
            wf_t[k] = wf

        nc.tensor.matmul(rgbps[:], identh[:], wf_t[nd - 1][:],
                         start=False, stop=True, skip_group_check=True)

        # ---- cross-core reduce (fp16 AllReduce) ----
        rgbh = epil.tile([128, 512], dhalf, tag="rgbh")
        nc.vector.tensor_copy(rgbh[:], rgbps[:])
        nc.sync.dma_start(cc_in[:].rearrange("(b p) y -> p b y", p=128),
                          rgbh[:].rearrange("p (b y) -> p b y", b=2))
        if sim:
            nc.sync.dma_start(cc_out[:], cc_in[:])
        else:
            nc.gpsimd.collective_compute(
                "AllReduce", ALU.add, replica_groups=[list(range(N_CORES))],
                ins=[cc_in[:]], outs=[cc_out[:]])
        rgbfh = epil.tile([128, 512], dhalf, tag="rgbfh")
        nc.sync.dma_start(rgbfh[:].rearrange("p (b y) -> p b y", b=2),
                          cc_out[:].rearrange("(b p) y -> p b y", p=128))

        # ---- normalization: global min/max/sum/sumsq then affine ----
        # ACT converts fp16->f32 and accumulates row-sums in one pass;
        # a second ACT pass accumulates row-sums of squares.  DVE does
        # min/max concurrently on the fp16 image.
        r4 = epil.tile([128, 4], dt, tag="r4")
        rgbf = epil.tile([128, 512], dt, tag="rgbf")
        nc.scalar.activation(rgbf[:], rgbfh[:], AF.Copy, accum_out=r4[:, 2:3])
        sqj = epil.tile([128, 512], dh, tag="sqj")
        nc.scalar.activation(sqj[:], rgbfh[:], AF.Square, accum_out=r4[:, 3:4])
        nc.vector.tensor_reduce(r4[:, 0:1], rgbfh[:], axis=AX, op=ALU.min)
        nc.vector.tensor_reduce(r4[:, 1:2], rgbfh[:], axis=AX, op=ALU.max)
        nc.vector.tensor_scalar_mul(r4[:, 1:2], r4[:, 1:2], -1.0)   # -max
        # transpose [min,-max] rows to partitions 0:2 via matmul, then a
        # single MIN reduce over the free dim gives [min, -max] per row
        trt = pst1.tile([128, 512], dt, tag="t1")
        tr = trt[0:2, 0:128]
        nc.tensor.matmul(tr, r4[:, 0:2], identf[:], start=True, stop=True)
        s2 = epil.tile([2, 1], dt, tag="s2")
        nc.vector.tensor_reduce(s2[0:2, 0:1], tr, axis=AX, op=ALU.min)
        # gather stats into one partition: gm = [min, -max, sum, sumsq]
        g4t = pst1.tile([128, 512], dt, tag="t1")
        g4 = g4t[0:1, 0:4]
        nc.tensor.matmul(g4[0:1, 0:2], s2[0:2, 0:1], identf[0:2, 0:2],
                         start=True, stop=True, skip_group_check=True)
        nc.tensor.matmul(g4[0:1, 2:4], onesc[:, 0:1], r4[:, 2:4],
                         start=True, stop=True, skip_group_check=True)
        gs = epil.tile([1, 4], dt, tag="gs")
        nc.scalar.activation(gs[:], g4, AF.Copy)
        # scalar chain on partition 0:  (see derivation in baseline)
        n = float(IMG * IMG)
        w = epil.tile([1, 8], dt, tag="w")
        nc.vector.tensor_mul(w[:, 0:1], gs[:, 2:3], gs[:, 2:3])      # sum^2
        nc.vector.tensor_scalar_mul(w[:, 1:2], w[:, 0:1], 1.0 / n)
        nc.vector.tensor_sub(w[:, 2:3], gs[:, 3:4], w[:, 1:2])
        nc.vector.tensor_scalar_mul(w[:, 3:4], w[:, 2:3], 1.0 / (n - 1.0))  # var
        nc.scalar.activation(w[:, 4:5], w[:, 3:4], AF.Sqrt)              # std
        nc.vector.tensor_scalar(w[:, 5:6], w[:, 4:5], EPS, EPS * EPS,
                                ALU.mult, ALU.add)                       # c
        sc2 = epil.tile([1, 2], dt, tag="sc2")
        nc.vector.tensor_sub(sc2[:, 0:1], w[:, 5:6], gs[:, 0:1])      # c - min
        nc.vector.tensor_add(w[:, 6:7], gs[:, 0:1], gs[:, 1:2])     # min-max
        nc.vector.tensor_sub(w[:, 7:8], w[:, 5:6], w[:, 6:7])           # c+max-min
        nc.vector.reciprocal(sc2[:, 1:2], w[:, 7:8])                    # inv
        # broadcast [1,2] -> [128,2]
        bct = psum.tile([128, 512], dt, tag="vb")
        bc = bct[:, 0:2]
        nc.tensor.matmul(bc, ones1[:], sc2[:], start=True, stop=True)
        bcs = epil.tile([128, 2], dt, tag="bcs")
        nc.scalar.activation(bcs[:], bc, AF.Copy)
        outsb = epil.tile([128, 512], dt, tag="outsb")
        nc.vector.tensor_scalar(outsb[:], rgbf[:], bcs[:, 0:1], bcs[:, 1:2],
                                ALU.add, ALU.mult)
        nc.sync.dma_start(out_d[:].rearrange("(b p) y -> p b y", p=128),
                          outsb[:].rearrange("p (b y) -> p b y", b=2))
    return nc


# ----------------------------------------------------------------------------
# entry points
# ----------------------------------------------------------------------------

def _axis_aligned(R, T):
    return (np.allclose(np.asarray(R[0]), np.eye(3), atol=1e-6)
            and abs(float(T[0][0]) - float(T[0][1])) < 1e-12)


class _CachedSpmd:
    """Compile the PJRT executable once; repeat calls only transfer + exec."""

    def __init__(self, nc, n_cores):
        import jax
        from concourse import mybir
        from concourse.bass2jax import (_bass_exec_p, install_neuronx_cc_hook,
                                        partition_id_tensor)
        from jax.experimental.shard_map import shard_map
        from jax.sharding import Mesh, PartitionSpec
        install_neuronx_cc_hook()
        self.jax = jax
        self.n_cores = n_cores
        pname = nc.partition_id_tensor.name if nc.partition_id_tensor else None
        in_names, out_names, out_avals, zero_outs = [], [], [], []
        for alloc in nc.m.functions[0].allocations:
            if not isinstance(alloc, mybir.MemoryLocationSet):
                continue
            name = alloc.memorylocations[0].name
            if alloc.kind == "ExternalInput":
                if name != pname:
                    in_names.append(name)
            elif alloc.kind == "ExternalOutput":
                shape = tuple(alloc.tensor_shape)
                dtype = mybir.dt.np(alloc.dtype)
                out_names.append(name)
                out_avals.append(jax.core.ShapedArray(shape, dtype))
                zero_outs.append(np.zeros(shape, dtype))
        self.in_names, self.out_names = in_names, out_names
        self.out_avals, self.zero_outs = out_avals, zero_outs
        n_params, n_outs = len(in_names), len(out_names)
        all_in = list(in_names) + list(out_names)
        if pname is not None:
            all_in.append(pname)

        def _body(*args):
            operands = list(args)
            if pname is not None:
                operands.append(partition_id_tensor())
            outs = _bass_exec_p.bind(
                *operands, out_avals=tuple(out_avals), in_names=tuple(all_in),
                out_names=tuple(out_names), lowering_input_output_aliases=(),
                sim_require_finite=True, sim_require_nnan=True, nc=nc)
            return tuple(outs)

        devices = jax.devices()[:n_cores]
        mesh = Mesh(np.asarray(devices), ("core",))
        in_specs = (PartitionSpec("core"),) * (n_params + n_outs)
        out_specs = (PartitionSpec("core"),) * n_outs
        self.fn = jax.jit(shard_map(_body, mesh=mesh, in_specs=in_specs,
                                    out_specs=out_specs, check_rep=False),
                          keep_unused=True)
        self._dev_zeros = [jax.device_put(np.zeros(
            (n_cores * z.shape[0], *z.shape[1:]), z.dtype)) for z in zero_outs]

    def run(self, in_maps):
        jax = self.jax
        concat = [np.concatenate([np.asarray(in_maps[c][nm])
                                  for c in range(self.n_cores)], axis=0)
                  for nm in self.in_names]
        outs = self.fn(*concat, *self._dev_zeros)
        jax.block_until_ready(outs)
        return [{nm: np.asarray(outs[i]).reshape(
                    self.n_cores, *self.out_avals[i].shape)[c]
                 for i, nm in enumerate(self.out_names)}
                for c in range(self.n_cores)]


_RUNNER_CACHE = {}


def _run(image3d, R, T, trace=False):
    vol = np.ascontiguousarray(np.asarray(image3d, np.float32)[0, 0])
    in_maps, nd = _host_inputs(vol, np.asarray(T, np.float64)[0])
    if nd not in _NC_CACHE:
        nc = _build_nc(nd)
        nc.finalize()
        _NC_CACHE[nd] = nc
    nc = _NC_CACHE[nd]
    if id(nc) not in _RUNNER_CACHE:
        _RUNNER_CACHE[id(nc)] = _CachedSpmd(nc, N_CORES)
    results = _RUNNER_CACHE[id(nc)].run(in_maps)
    out = np.asarray(results[0]["out"], np.float32)[None, None]
    return out, results


def _numpy_fallback(image3d, R, T):
    """Direct port of the reference for non-axis-aligned cameras."""
    image3d = np.asarray(image3d, np.float32)
    R = np.asarray(R, np.float32); T = np.asarray(T, np.float32)
    B, C, D, H, W = image3d.shape
    vol = image3d[:, 0]
    vox = 3.0 / max(C, D)
    yg, xg = np.meshgrid(np.linspace(-1, 1, IMG), np.linspace(-1, 1, IMG),
                         indexing='ij')
    depths = np.linspace(MIN_D, MAX_D, NPTS)
    pcam = np.stack([xg[..., None] * depths / FOCAL,
                     yg[..., None] * depths / FOCAL,
                     np.broadcast_to(depths, (IMG, IMG, NPTS))], -1)
    v = pcam[None] - T[:, None, None, None, :]
    pw = np.einsum('bhwpj,bkj->bhwpk', v, R)
    half = np.array([vox * (W - 1) / 2, vox * (H - 1) / 2, vox * (D - 1) / 2])
    local = pw / half

    def tri(voln, pts):
        ix = (pts[..., 0] + 1) * .5 * (W - 1)
        iy = (pts[..., 1] + 1) * .5 * (H - 1)
        iz = (pts[..., 2] + 1) * .5 * (D - 1)
        out = np.zeros(ix.shape, np.float32)
        x0, y0, z0 = np.floor(ix), np.floor(iy), np.floor(iz)
        fx, fy, fz = ix - x0, iy - y0, iz - z0
        for zi, wz in ((z0, 1 - fz), (z0 + 1, fz)):
            for yi, wy in ((y0, 1 - fy), (y0 + 1, fy)):
                for xi, wx in ((x0, 1 - fx), (x0 + 1, fx)):
                    valid = ((xi >= 0) & (xi < W) & (yi >= 0) & (yi < H)
                             & (zi >= 0) & (zi < D))
                    vv = voln[np.clip(zi, 0, D - 1).astype(int),
                              np.clip(yi, 0, H - 1).astype(int),
                              np.clip(xi, 0, W - 1).astype(int)]
                    out += np.where(valid, vv * (wz * wy * wx), 0).astype(np.float32)
        return out

    feat = np.stack([tri(vol[b], local[b]) for b in range(B)])
    sigma = DENSITY * np.stack([tri(np.ones((D, H, W), np.float32), local[b])
                                for b in range(B)])
    t = (1.0 + 1e-10) - sigma
    ab = np.cumprod(t, -1)
    ab = np.concatenate([np.ones_like(ab[..., :1]), ab[..., :-1]], -1)
    rgb = np.sum(sigma * ab * feat, -1)
    out = np.transpose(rgb, (0, 2, 1))[:, None]
    s = (out - out.mean()) / (np.std(out, ddof=1) + EPS)
    return ((s - s.min() + EPS) / (s.max() - s.min() + EPS)).astype(np.float32)


def kernel(image3d, R, T):
    if not _axis_aligned(R, T):
        return _numpy_fallback(image3d, R, T)
    out, _ = _run(image3d, R, T, trace=False)
    return out
